# revision 22
# baseline (speedup 1.0000x reference)
"""Multi-head attention kernel for Trainium2, SPMD over 8 NeuronCores.

Problem: qkv (8, 1536, 2048) f32 -> out (8, 512, 2048) f32
  B=8 batches, H=8 heads, C=64 channels/head, T=2048 tokens.
  out[b] = concat_h( softmax((q_h*s)^T (k_h*s)) applied to v_h )
  with s = C**-0.25 (i.e. scores scaled by C**-0.5 overall).

Sharding: batch b -> core b. Each core computes 8 heads independently;
no collectives needed.

Structure: flash-style t-block tiling. Each head is processed as four
independent t-blocks of 512 columns; a t-block runs 16 key-chunk
iterations of [QK matmul -> exp -> AV matmul] where every PSUM tile is
exactly one 2KB bank. That makes the AV accumulator 1 bank (x2
rotation) and leaves SIX banks for score-tile rotation, which
decouples the PE from the exp engines deeply enough that the PE
stream (one 512-col QK + one 512-col AV matmul per slot, uniform
across all head/pair boundaries) never stalls on softmax latency --
recurring stalls reset the tensor engine's p-state ramp and halve its
clock, which dominated all earlier layouts.

Engine assignment:
  - PE: QK (k-chunk stationary, q-block moving) and AV (vt stationary
    = [v^T | ones] so PSUM row 64 accumulates the softmax denominator).
  - exp: even chunks on ACT (exact Exp), odd chunks on DVE
    (Schraudolph: one tensor_scalar writing bf16 bits via an int16
    convert; softmax normalization cancels its common-mode bias;
    end-to-end rel err ~1.3e-2 < 2e-2).
  - v^T via XBAR DMA-transpose of a bf16 v copy into a contiguous
    tile, bounced by plain DMA into the 65-pitch vt (strided XBAR
    output and split/80-partition XBAR variants are wrong on hw).
  - casts f32->bf16 on ACT (free-dim split only; partitions are
    parallel); per-block tails: evac on ACT, l copy + broadcast +
    normalize multiply + output DMA on gpsimd, reciprocal on DVE
    (custom DVE ops cannot partition-shift, so l is staged to
    partition 0 by the gpsimd copy first).
  - input DMAs all issued up front on the SP queue; output DMAs from
    gpsimd.
Emission-order rules (the tile framework tracks dependencies online,
so readers must be emitted after all their writers and before any
clobbering writer): a block's tail tasks enter the queue only when its
last AV has been emitted; pair prep runs during the odd head of the
previous pair, long after the buffers' previous readers were emitted.
"""

import os
import sys
from collections import deque

import numpy as np

for _p in ("/opt/trn_rl_repo", "/root/.axon_site/_ro/trn_rl_repo"):
    if os.path.isdir(_p) and _p not in sys.path:
        sys.path.insert(0, _p)

B, H, C, T = 8, 8, 64, 2048
HC = H * C  # 512
W = 3 * HC  # 1536
NCH = T // 128  # 16 key chunks of 128
TB = 512  # t-block width
NB = T // TB  # 4 t-blocks per head
TH = 1024

# Schraudolph exp writing bf16 bits via an int16 convert:
# bits = round(0.125*s*128*log2(e) + 16256 + c); +0.5 makes a
# truncating convert round; c centers the sawtooth.
SCH_A = 0.125 * 128 * 1.4426950408889634
SCH_B = 16256.0 - 5.5 + 0.5

AV_LAG = int(os.environ.get("AV_LAG", "2"))
SC_BUFS = int(os.environ.get("SC_BUFS", "6"))

_CACHE = {}


def _build_nc():
    from contextlib import ExitStack

    import concourse.mybir as mybir
    from concourse import bacc
    from concourse.tile import TileContext

    f32 = mybir.dt.float32
    bf16 = mybir.dt.bfloat16
    i16 = mybir.dt.int16
    Exp = mybir.ActivationFunctionType.Exp
    mul_op = mybir.AluOpType.mult
    add_op = mybir.AluOpType.add

    nc = bacc.Bacc("TRN2", target_bir_lowering=False, debug=False)
    qkv = nc.declare_dram_parameter("qkv", [W, T], f32, isOutput=False)
    out = nc.declare_dram_parameter("out", [HC, T], f32, isOutput=True)

    with TileContext(nc) as tc, ExitStack() as ctx:
        qkv_pool = ctx.enter_context(tc.tile_pool(name="qkvp", bufs=2))
        vt_pool = ctx.enter_context(tc.tile_pool(name="vtp", bufs=2))
        pt_pool = ctx.enter_context(tc.tile_pool(name="ptp", bufs=10))
        out_pool = ctx.enter_context(tc.tile_pool(name="outp", bufs=2))
        l_pool = ctx.enter_context(tc.tile_pool(name="lp", bufs=2))
        ps_sc = ctx.enter_context(
            tc.tile_pool(name="ps_sc", bufs=SC_BUFS, space="PSUM")
        )
        ps_av = ctx.enter_context(tc.tile_pool(name="ps_av", bufs=2, space="PSUM"))

        pairs = []
        for p in range(4):
            pr = {}
            pr["q2"] = qkv_pool.tile([128, T], f32, tag="q2", name="q2")
            pr["k2"] = qkv_pool.tile([128, T], f32, tag="k2", name="k2")
            pr["v2"] = qkv_pool.tile([128, T], f32, tag="v2", name="v2")
            pr["q2h"] = qkv_pool.tile([128, T], bf16, tag="q2h", name="q2h")
            pr["k2h"] = qkv_pool.tile([128, T], bf16, tag="k2h", name="k2h")
            pr["v2h"] = qkv_pool.tile([128, T], bf16, tag="v2h", name="v2h")
            pr["vt"] = [
                vt_pool.tile([128, NCH, 65], bf16, tag=f"vt{i}", name=f"vt{i}")
                for i in range(2)
            ]
            pr["vtt"] = [
                vt_pool.tile([128, NCH, 64], bf16, tag=f"vtt{i}", name=f"vtt{i}")
                for i in range(2)
            ]
            pairs.append(pr)

        # ---- all input DMAs up front, in consumption order
        for p, pr in enumerate(pairs):
            r0 = p * 128
            if p == 0:
                nc.sync.dma_start(out=pr["k2"][0:64, 0:128], in_=qkv[HC : HC + 64, 0:128])
                nc.sync.dma_start(out=pr["q2"][0:64, 0:TB], in_=qkv[0:64, 0:TB])
                nc.sync.dma_start(out=pr["v2"], in_=qkv[2 * HC : 2 * HC + 128, :])
                nc.sync.dma_start(out=pr["k2"][0:64, 128:T], in_=qkv[HC : HC + 64, 128:T])
                nc.sync.dma_start(out=pr["q2"][0:64, TB:T], in_=qkv[0:64, TB:T])
                nc.sync.dma_start(out=pr["k2"][64:128, :], in_=qkv[HC + 64 : HC + 128, :])
                nc.sync.dma_start(out=pr["q2"][64:128, :], in_=qkv[64:128, :])
            else:
                nc.sync.dma_start(out=pr["k2"], in_=qkv[HC + r0 : HC + r0 + 128, :])
                nc.sync.dma_start(out=pr["q2"], in_=qkv[r0 : r0 + 128, :])
                nc.sync.dma_start(out=pr["v2"], in_=qkv[2 * HC + r0 : 2 * HC + r0 + 128, :])

        def emit_prep_tasks(p, staged):
            pr = pairs[p]
            k2, q2, v2 = pr["k2"], pr["q2"], pr["v2"]
            k2h, q2h, v2h = pr["k2h"], pr["q2h"], pr["v2h"]
            vts, vtts = pr["vt"], pr["vtt"]
            tasks = [
                lambda: nc.vector.memset(vts[0][:, :, 64:65], 1.0),
                lambda: nc.vector.memset(vts[1][:, :, 64:65], 1.0),
                lambda: nc.scalar.copy(v2h[:, 0:TH], v2[:, 0:TH]),
                lambda: nc.scalar.copy(v2h[:, TH:T], v2[:, TH:T]),
            ]

            def trans(i):
                def f():
                    nc.scalar.dma_start_transpose(
                        vtts[i], v2h[i * 64 : i * 64 + 64, :]
                    )
                    nc.scalar.dma_start(out=vts[i][:, :, 0:64], in_=vtts[i])

                return f

            tasks += [trans(0), trans(1)]
            if staged:
                tasks += [
                    lambda: nc.scalar.copy(k2h[0:64, 128:T], k2[0:64, 128:T]),
                    lambda: nc.scalar.copy(q2h[0:64, TB:T], q2[0:64, TB:T]),
                    lambda: nc.scalar.copy(k2h[64:128, :], k2[64:128, :]),
                    lambda: nc.scalar.copy(q2h[64:128, :], q2[64:128, :]),
                ]
            else:
                for m in range(2):
                    t0, t1 = m * TH, (m + 1) * TH
                    tasks += [
                        lambda t0=t0, t1=t1: nc.scalar.copy(k2h[:, t0:t1], k2[:, t0:t1]),
                        lambda t0=t0, t1=t1: nc.scalar.copy(q2h[:, t0:t1], q2[:, t0:t1]),
                    ]
            return tasks

        # pair 0 startup: the two pieces QK(block 0, chunk 0) needs, then
        # the full prep
        pr0 = pairs[0]
        nc.scalar.copy(pr0["k2h"][0:64, 0:128], pr0["k2"][0:64, 0:128])
        nc.scalar.copy(pr0["q2h"][0:64, 0:TB], pr0["q2"][0:64, 0:TB])
        for t in emit_prep_tasks(0, staged=True):
            t()

        def emit_tail_tasks(h, b, av, av_sb, l_sb, rl, rlb, o_sb):
            t0, t1 = b * TB, (b + 1) * TB

            def evac():
                nc.scalar.copy(av_sb[0:65, t0:t1], av[0:65, :])

            def lcopy():
                # partition 64 -> 0 on a plain copy (custom DVE ops
                # cannot partition-shift on hw)
                nc.gpsimd.tensor_copy(l_sb[:, t0:t1], av_sb[64:65, t0:t1])

            def recip():
                nc.vector.reciprocal_approx_fast(
                    out=rl[:, t0:t1], in_=l_sb[:, t0:t1]
                )

            def bcast():
                nc.gpsimd.partition_broadcast(rlb[:, t0:t1], rl[:, t0:t1])

            def mult_dma():
                nc.gpsimd.tensor_mul(
                    o_sb[:, t0:t1], av_sb[0:64, t0:t1], rlb[:, t0:t1]
                )
                nc.gpsimd.dma_start(
                    out=out[h * 64 : (h + 1) * 64, t0:t1], in_=o_sb[:, t0:t1]
                )

            return [evac, lcopy, recip, bcast, mult_dma]

        pending_av = deque()  # (closure, tail_closure_list_or_None)
        tail_bg = deque()
        bg = deque()

        for h in range(H):
            p = h // 2
            o = (h % 2) * 64
            pr = pairs[p]
            q = pr["q2h"][o : o + 64, :]
            k = pr["k2h"][o : o + 64, :]
            vt = pr["vt"][h % 2]

            av_sb = out_pool.tile([65, T], f32, tag="avsb", name="avsb")
            l_sb = l_pool.tile([1, T], f32, tag="lsb", name="lsb")
            rl = l_pool.tile([1, T], f32, tag="rl", name="rl")
            rlb = l_pool.tile([64, T], f32, tag="rlb", name="rlb")
            o_sb = out_pool.tile([64, T], f32, tag="osb", name="osb")

            if h % 2 == 1 and p + 1 < 4:
                bg.extend(emit_prep_tasks(p + 1, staged=False))

            for b in range(NB):
                t0 = b * TB
                av = ps_av.tile([128, TB], f32, tag="av", name="av")
                tail = emit_tail_tasks(h, b, av, av_sb, l_sb, rl, rlb, o_sb)
                for j in range(NCH):
                    sc = ps_sc.tile([128, TB], f32, tag="sc", name="sc")
                    nc.tensor.matmul(
                        sc,
                        k[:, j * 128 : (j + 1) * 128],
                        q[:, t0 : t0 + TB],
                        start=True,
                        stop=True,
                    )
                    pt = pt_pool.tile([128, TB], bf16, name="pt")
                    if j % 2 == 0:
                        nc.scalar.activation(pt, sc, Exp, scale=0.125)
                    else:
                        nc.vector.tensor_scalar(
                            pt.bitcast(i16), sc, SCH_A, SCH_B, mul_op, add_op
                        )

                    def av_emit(av=av, vt=vt, pt=pt, j=j):
                        nc.tensor.matmul(
                            av[0:65, :],
                            vt[:, j, :],
                            pt,
                            start=(j == 0),
                            stop=(j == NCH - 1),
                            skip_group_check=True,
                        )

                    pending_av.append((av_emit, tail if j == NCH - 1 else None))
                    if len(pending_av) > AV_LAG:
                        fn, tl = pending_av.popleft()
                        fn()
                        if tl is not None:
                            tail_bg.extend(tl)

                    if tail_bg:
                        tail_bg.popleft()()
                    if bg:
                        bg.popleft()()

        # flush
        while pending_av:
            fn, tl = pending_av.popleft()
            fn()
            if tl is not None:
                tail_bg.extend(tl)
        while tail_bg:
            tail_bg.popleft()()
        while bg:
            bg.popleft()()

    nc.finalize()
    return nc


def _get_nc():
    if "nc" not in _CACHE:
        _CACHE["nc"] = _build_nc()
    return _CACHE["nc"]


def _run(qkv_full, trace=False, tmpdir=None):
    """qkv_full: (8, 1536, 2048) f32. Returns (out (8,512,2048) f32, exec_ns)."""
    from concourse.bass_utils import run_bass_kernel_spmd

    nc = _get_nc()
    qkv_full = np.ascontiguousarray(np.asarray(qkv_full, dtype=np.float32))
    in_maps = [{"qkv": qkv_full[i]} for i in range(B)]
    res = run_bass_kernel_spmd(
        nc, in_maps, core_ids=list(range(B)), trace=trace, tmpdir=tmpdir
    )
    outs = np.stack([np.asarray(res.results[i]["out"]) for i in range(B)], axis=0)
    return outs, res.exec_time_ns


def kernel(qkv, n_heads=8):
    out, _ = _run(qkv)
    return out.astype(np.float32)


# revision 23
# speedup vs baseline: 1.9642x; 1.9642x over previous
"""Multi-head attention kernel for Trainium2, SPMD over 8 NeuronCores.

Problem: qkv (8, 1536, 2048) f32 -> out (8, 512, 2048) f32
  B=8 batches, H=8 heads, C=64 channels/head, T=2048 tokens.
  out[b] = concat_h( softmax((q_h*s)^T (k_h*s)) applied to v_h )
  with s = C**-0.25 (i.e. scores scaled by C**-0.5 overall).

Sharding: batch b -> core b. Each core computes 8 heads independently;
no collectives needed.

Per-head algorithm on one core (all on-chip):
  for each s-chunk (128 keys):
    scoresT[s,t] = sum_c k[c,s] q[c,t]        (PE, f32)
    pT[s,t] = exp(0.125 * scoresT)            (ACT, bf16 out, no max-sub:
                                               scores ~ N(0,1), safe in f32)
    acc[c,t] += vT_ones[s, c] * pT[s,t]       (PE, bf16; row 64 of vT_ones
                                               is ones -> acc[64,t] = l[t])
  out[c,t] = acc[c,t] / l[t]                  (DVE + DMA broadcast of 1/l)
"""

import os
import sys

import numpy as np

for _p in ("/opt/trn_rl_repo", "/root/.axon_site/_ro/trn_rl_repo"):
    if os.path.isdir(_p) and _p not in sys.path:
        sys.path.insert(0, _p)

B, H, C, T = 8, 8, 64, 2048
HC = H * C  # 512
W = 3 * HC  # 1536
NCH = T // 128  # 16 key chunks of 128
THALF = T // 2  # 1024

_CACHE = {}


def _build_nc():
    from contextlib import ExitStack

    import concourse.bass as bass
    import concourse.mybir as mybir
    from concourse import bacc
    from concourse.masks import make_identity
    from concourse.tile import TileContext

    f32 = mybir.dt.float32
    bf16 = mybir.dt.bfloat16
    Exp = mybir.ActivationFunctionType.Exp

    nc = bacc.Bacc("TRN2", target_bir_lowering=False, debug=False)
    qkv = nc.declare_dram_parameter("qkv", [W, T], f32, isOutput=False)
    out = nc.declare_dram_parameter("out", [HC, T], f32, isOutput=True)

    with TileContext(nc) as tc, ExitStack() as ctx:
        singles = ctx.enter_context(tc.tile_pool(name="singles", bufs=1))
        qkv_pool = ctx.enter_context(tc.tile_pool(name="qkvp", bufs=2))
        vt_pool = ctx.enter_context(tc.tile_pool(name="vtp", bufs=2))
        pt_pool = ctx.enter_context(tc.tile_pool(name="ptp", bufs=10))
        out_pool = ctx.enter_context(tc.tile_pool(name="outp", bufs=2))
        l_pool = ctx.enter_context(tc.tile_pool(name="lp", bufs=2))
        ps_sc = ctx.enter_context(tc.tile_pool(name="ps_sc", bufs=2, space="PSUM"))
        ps_av = ctx.enter_context(tc.tile_pool(name="ps_av", bufs=1, space="PSUM"))

        # identity for PE transposes, one copy per partition half so the
        # rhs base partition matches lhsT for both heads of a pair
        ident = singles.tile([128, 64], f32)
        make_identity(nc, ident[0:64, :])
        make_identity(nc, ident[64:128, :])


        for pair in range(4):
            q2 = qkv_pool.tile([128, T], f32, tag="q2")
            k2 = qkv_pool.tile([128, T], f32, tag="k2")
            v2 = qkv_pool.tile([128, T], f32, tag="v2")
            q2b = qkv_pool.tile([128, T], bf16, tag="q2b")
            k2b = qkv_pool.tile([128, T], bf16, tag="k2b")
            r0 = pair * 128
            if pair == 0:
                # load + cast only what QK_0/exp_0 need first (32KB of
                # k, half of q), then the rest: first exp fires earlier
                nc.sync.dma_start(out=k2[0:64, 0:128], in_=qkv[HC : HC + 64, 0:128])
                nc.sync.dma_start(out=q2[0:64, 0:THALF], in_=qkv[0:64, 0:THALF])
                nc.vector.tensor_copy(k2b[0:64, 0:128], k2[0:64, 0:128])
                nc.vector.tensor_copy(q2b[0:64, 0:THALF], q2[0:64, 0:THALF])
                nc.sync.dma_start(out=k2[0:64, 128:T], in_=qkv[HC : HC + 64, 128:T])
                nc.sync.dma_start(out=q2[0:64, THALF:T], in_=qkv[0:64, THALF:T])
                nc.vector.tensor_copy(k2b[0:64, 128:T], k2[0:64, 128:T])
                nc.vector.tensor_copy(q2b[0:64, THALF:T], q2[0:64, THALF:T])
                nc.sync.dma_start(out=v2, in_=qkv[2 * HC : 2 * HC + 128, :])
                nc.sync.dma_start(out=k2[64:128, :], in_=qkv[HC + 64 : HC + 128, :])
                nc.sync.dma_start(out=q2[64:128, :], in_=qkv[64:128, :])
                nc.vector.tensor_copy(k2b[64:128, :], k2[64:128, :])
                nc.vector.tensor_copy(q2b[64:128, :], q2[64:128, :])
            else:
                nc.sync.dma_start(out=q2, in_=qkv[r0 : r0 + 128, :])
                nc.sync.dma_start(out=k2, in_=qkv[HC + r0 : HC + r0 + 128, :])
                nc.sync.dma_start(
                    out=v2, in_=qkv[2 * HC + r0 : 2 * HC + r0 + 128, :]
                )
                # bf16 q/k: matmul streams 1 col/cycle vs 2 for f32
                nc.vector.tensor_copy(q2b, q2)
                nc.vector.tensor_copy(k2b, k2)

            for hh in range(2):
                h = pair * 2 + hh
                o = hh * 64
                q = q2b[o : o + 64, :]
                k = k2b[o : o + 64, :]
                v = v2[o : o + 64, :]

                # v -> [s, c] transposes happen on the PE, but are emitted
                # inside chunks 0-1 below so they sit in PE slack during the
                # first exps instead of blocking QK_0 at the head boundary
                trans = ps_av.tile([128, NCH, 64], f32, tag="av")
                vt = vt_pool.tile([128, NCH, 66], bf16)

                def emit_trans(lo, hi):
                    for j in range(lo, hi):
                        nc.tensor.transpose(
                            trans[:, j, :],
                            v[:, j * 128 : (j + 1) * 128],
                            ident[o : o + 64, :],
                        )
                    if hi == NCH:
                        # vt rows: 0..63 = v, 64 = ones (l via the AV matmul)
                        nc.vector.tensor_copy(vt[:, :, 0:64], trans)
                        nc.vector.memset(vt[:, :, 64:65], 1.0)

                av = ps_av.tile([128, T], f32, tag="av")

                def emit_av(j, pts_j):
                    # one LDW for all 4 AV matmuls of chunk j
                    for half in range(2):
                        t0 = half * THALF
                        for qq in range(2):
                            nc.tensor.matmul(
                                av[0:65, t0 + qq * 512 : t0 + (qq + 1) * 512],
                                vt[:, j, 0:65],
                                pts_j[half][:, qq * 512 : (qq + 1) * 512],
                                start=(j == 0),
                                stop=(j == NCH - 1),
                                skip_group_check=True,
                            )

                # software pipeline: QK(j)+exp(j) stream, AV lags one chunk
                # so the PE can run QK(j+1) between exp(j,lo) and exp(j,hi)
                prev_pts = None
                for j in range(NCH):
                    kj = k[:, j * 128 : (j + 1) * 128]
                    scs = []
                    for half in range(2):
                        t0 = half * THALF
                        sc = ps_sc.tile([128, THALF], f32, tag="sc")
                        scs.append(sc)
                        for qq in range(2):
                            nc.tensor.matmul(
                                sc[:, qq * 512 : (qq + 1) * 512],
                                kj,
                                q[:, t0 + qq * 512 : t0 + (qq + 1) * 512],
                                start=True,
                                stop=True,
                            )
                    pts = []
                    for half in range(2):
                        pt = pt_pool.tile([128, THALF], bf16)
                        pts.append(pt)
                        nc.scalar.activation(pt, scs[half], Exp, scale=0.125)
                    if j == 0:
                        emit_trans(0, NCH // 2)
                    elif j == 1:
                        emit_trans(NCH // 2, NCH)
                    if prev_pts is not None:
                        emit_av(j - 1, prev_pts)
                    prev_pts = pts
                emit_av(NCH - 1, prev_pts)

                # evacuate av to SBUF promptly (two halves so the slot frees
                # incrementally); normalize happens off the critical path
                av_sb = out_pool.tile([65, T], f32, tag="avsb")
                nc.vector.tensor_copy(av_sb[:, 0:THALF], av[0:65, 0:THALF])
                nc.vector.tensor_copy(av_sb[:, THALF:T], av[0:65, THALF:T])
                # normalize out = av[0:64] * (1/l), l = av row 64; done in
                # t-halves so each chain starts as soon as its evac half
                # lands (shortens the kernel tail); l staged to partition 0
                # on idle gpsimd (partition_broadcast reads partition 0)
                l_sb = l_pool.tile([1, T], f32, tag="lsb")
                l_bc = l_pool.tile([64, T], f32, tag="lbc")
                rl = l_pool.tile([64, T], f32, tag="rl")
                o_sb = out_pool.tile([64, T], f32, tag="osb")
                for half in range(2):
                    t0, t1 = half * THALF, (half + 1) * THALF
                    nc.gpsimd.tensor_copy(l_sb[:, t0:t1], av_sb[64:65, t0:t1])
                    nc.gpsimd.partition_broadcast(l_bc[:, t0:t1], l_sb[:, t0:t1])
                    nc.vector.reciprocal_approx_fast(
                        out=rl[:, t0:t1], in_=l_bc[:, t0:t1]
                    )
                    nc.vector.tensor_mul(
                        o_sb[:, t0:t1], av_sb[0:64, t0:t1], rl[:, t0:t1]
                    )
                    nc.sync.dma_start(
                        out=out[h * 64 : (h + 1) * 64, t0:t1], in_=o_sb[:, t0:t1]
                    )

    nc.finalize()
    return nc


def _get_nc():
    if "nc" not in _CACHE:
        _CACHE["nc"] = _build_nc()
    return _CACHE["nc"]


def _run(qkv_full, trace=False, tmpdir=None):
    """qkv_full: (8, 1536, 2048) f32. Returns (out (8,512,2048) f32, exec_ns)."""
    from concourse.bass_utils import run_bass_kernel_spmd

    nc = _get_nc()
    qkv_full = np.ascontiguousarray(np.asarray(qkv_full, dtype=np.float32))
    in_maps = [{"qkv": qkv_full[i]} for i in range(B)]
    res = run_bass_kernel_spmd(
        nc, in_maps, core_ids=list(range(B)), trace=trace, tmpdir=tmpdir
    )
    outs = np.stack([np.asarray(res.results[i]["out"]) for i in range(B)], axis=0)
    return outs, res.exec_time_ns


def kernel(qkv, n_heads=8):
    out, _ = _run(qkv)
    return out.astype(np.float32)

